# revision 8
# baseline (speedup 1.0000x reference)
"""Multi-head causal attention with RoPE on 8 TRN2 NeuronCores.

Problem: B=2, T=2048, D=1024, H=16 heads, head_dim=64.
  out = softmax(mask(rope(x@Wq.T) @ rope(x@Wk.T).T / 8)) @ (x@Wv.T) @ Wo.T

Sharding: tensor-parallel over heads. Core c owns heads {2c, 2c+1}:
  - computes Q/K/V projections for its 2 heads over all 4096 tokens
    (x pre-transposed on host to [1024, 4096], weights pre-sliced and
    pre-transposed; the 1/sqrt(hd) scale is folded into Wq),
  - RoPE via a block-diagonal rotation matmul + cos/sin elementwise,
  - causal flash-style attention in transposed layout (scores^T [k, q]
    tiles, exp on ScalarE, lower-triangle tiles only, row-sums via an
    appended ones-column on V),
  - an AllToAll (2 x 1MB, split by local head) redistributes attention
    outputs from head-sharded to row-sharded,
  - row-parallel output projection: core c computes output rows
    [512c, 512(c+1)) of the flattened [4096, 1024] output.

All matmuls run as float32r (fp32 storage, TF32-like multiply, full PE
rate at N>=256); accumulation is fp32 in PSUM.
"""
import sys

sys.path.insert(0, "/opt/trn_rl_repo")

import numpy as np

from concourse import bacc, mybir, tile
from concourse import bass_utils

N_CORES = 8
B, T, D, H = 2, 2048, 1024, 16
HD = D // H              # 64
HPC = H // N_CORES       # 2 heads per core
BT = B * T               # 4096
NF = D // 128            # 8 feature chunks
NTC = BT // 512          # 8 t-chunks of 512
QCHUNK = 512
ROWS_PER_CORE = BT // N_CORES  # 512 output rows per core

F32 = mybir.dt.float32
F32R = mybir.dt.float32r
BF16 = mybir.dt.bfloat16

_CACHE = {}


def _rot_matrix():
    """R2 = blockdiag(R, R), R@u = rotate_half(u) per 64-dim head."""
    half = HD // 2
    R = np.zeros((HD, HD), dtype=np.float32)
    for i in range(half):
        R[i, i + half] = -1.0
        R[i + half, i] = 1.0
    R2 = np.zeros((2 * HD, 2 * HD), dtype=np.float32)
    R2[:HD, :HD] = R
    R2[HD:, HD:] = R
    return R2


def build(debug=False):
    nc = bacc.Bacc("TRN2", target_bir_lowering=False, debug=False,
                   num_devices=N_CORES)

    # ---- DRAM parameters (per-core shards, host-prepped layouts) ----
    xt = nc.declare_dram_parameter("xt", [D, BT], F32, isOutput=False)
    wq_t = nc.declare_dram_parameter("wq_t", [D, 128], F32, isOutput=False)
    wk_t = nc.declare_dram_parameter("wk_t", [D, 128], F32, isOutput=False)
    wv_t = nc.declare_dram_parameter("wv_t", [D, 128], F32, isOutput=False)
    wo_t = nc.declare_dram_parameter("wo_t", [D, D], F32, isOutput=False)
    cos2 = nc.declare_dram_parameter("cos2", [128, T], F32, isOutput=False)
    sin2 = nc.declare_dram_parameter("sin2", [128, T], F32, isOutput=False)
    rot2t = nc.declare_dram_parameter("rot2t", [128, 128], F32, isOutput=False)
    ident = nc.declare_dram_parameter("ident", [128, 128], F32, isOutput=False)
    trimask = nc.declare_dram_parameter("trimask", [128, 128], F32, isOutput=False)
    out = nc.declare_dram_parameter("out", [ROWS_PER_CORE, D], F32, isOutput=True)
    if debug:
        dbg_qrope = nc.declare_dram_parameter("dbg_qrope", [128, BT], F32, isOutput=True)
        dbg_krope = nc.declare_dram_parameter("dbg_krope", [128, BT], F32, isOutput=True)
        dbg_vall = nc.declare_dram_parameter("dbg_vall", [128, B * HPC * 16 * 65], F32, isOutput=True)
        dbg_attout = nc.declare_dram_parameter("dbg_attout", [64, HPC * BT], F32, isOutput=True)
        dbg_attall = nc.declare_dram_parameter("dbg_attall", [128, N_CORES * QCHUNK], F32, isOutput=True)

    with tile.TileContext(nc) as tc, nc.allow_low_precision(reason="f32r compute"):
        with (
            tc.tile_pool(name="consts", bufs=1) as cpool,
            tc.tile_pool(name="work", bufs=2) as wpool,
            tc.tile_pool(name="psum", bufs=1, space="PSUM") as ppool,
            tc.tile_pool(name="dram", bufs=1, space="DRAM") as dpool,
        ):
            # ---- load constants ----
            wq_sb = cpool.tile([128, NF, 128], F32R, tag="wq")
            wk_sb = cpool.tile([128, NF, 128], F32R, tag="wk")
            wv_sb = cpool.tile([128, NF, 128], F32R, tag="wv")
            for w_sb, w_dram in ((wq_sb, wq_t), (wk_sb, wk_t), (wv_sb, wv_t)):
                nc.sync.dma_start(
                    w_sb[:],
                    w_dram[:].rearrange("(c p) m -> c p m", p=128)
                    .transpose([1, 0, 2]).bitcast(F32R),
                )
            rot_sb = cpool.tile([128, 128], F32R, tag="rot")
            nc.sync.dma_start(rot_sb[:], rot2t[:].bitcast(F32R))
            id_sb = cpool.tile([128, 128], F32R, tag="ident")
            nc.sync.dma_start(id_sb[:], ident[:].bitcast(F32R))
            tri_sb = cpool.tile([128, 128], F32R, tag="tri")
            nc.sync.dma_start(tri_sb[:], trimask[:].bitcast(F32R))
            cos_sb = cpool.tile([128, T], F32R, tag="cos")
            nc.sync.dma_start(cos_sb[:], cos2[:].bitcast(F32R))
            sin_sb = cpool.tile([128, T], F32R, tag="sin")
            nc.sync.dma_start(sin_sb[:], sin2[:].bitcast(F32R))

            # persistent activations
            qrope = cpool.tile([128, BT], F32R, tag="qrope")
            krope = cpool.tile([128, BT], F32R, tag="krope")
            # V per (b, h): [128 t-part, 16 t-tiles, 65] (col 64 = ones)
            v_all = cpool.tile([128, B, HPC, T // 128, 65], F32R, tag="v_all")
            nc.vector.memset(v_all[:, :, :, :, 64].bitcast(F32), 1.0)
            # attention output, head-major [64, B*T] per local head
            att_out = cpool.tile([64, HPC, BT], BF16, tag="att_out")
            # post-A2A gathered att rows for my q-range [128, 8 src, 512]
            att_all = cpool.tile([128, N_CORES, QCHUNK], BF16, tag="att_all")

            # ---- Phase A: projections + RoPE + V transpose ----
            for j in range(NTC):
                b = j // 4
                tl = (j % 4) * 512            # t offset within batch
                J = slice(j * 512, (j + 1) * 512)
                TL = slice(tl, tl + 512)

                xt_sb = wpool.tile([128, NF, 512], F32R, tag="xt")
                nc.sync.dma_start(
                    xt_sb[:],
                    xt[:, J].rearrange("(c p) t -> c p t", p=128)
                    .transpose([1, 0, 2]).bitcast(F32R),
                )

                ps_q = ppool.tile([128, 512], F32, tag="pA")
                ps_k = ppool.tile([128, 512], F32, tag="pB")
                ps_v = ppool.tile([128, 512], F32, tag="pC")
                for f in range(NF):
                    st, sp = (f == 0), (f == NF - 1)
                    nc.tensor.matmul(ps_q[:], wq_sb[:, f, :], xt_sb[:, f, :],
                                     start=st, stop=sp)
                    nc.tensor.matmul(ps_k[:], wk_sb[:, f, :], xt_sb[:, f, :],
                                     start=st, stop=sp)
                    nc.tensor.matmul(ps_v[:], wv_sb[:, f, :], xt_sb[:, f, :],
                                     start=st, stop=sp)

                # copy projections to SBUF
                qT = wpool.tile([128, 512], F32R, tag="qT")
                kT = wpool.tile([128, 512], F32R, tag="kT")
                vT = wpool.tile([128, 512], F32R, tag="vT")
                nc.scalar.copy(qT[:], ps_q[:])
                nc.scalar.copy(kT[:], ps_k[:])
                nc.scalar.copy(vT[:], ps_v[:])

                # RoPE: rope(u) = u*cos + (R2@u)*sin
                for src, dst in ((qT, qrope), (kT, krope)):
                    ps_r = ppool.tile([128, 512], F32, tag="pD")
                    nc.tensor.matmul(ps_r[:], rot_sb[:], src[:],
                                     start=True, stop=True)
                    tmp = wpool.tile([128, 512], F32R, tag="ropetmp")
                    nc.vector.tensor_mul(tmp[:], ps_r[:], sin_sb[:, TL])
                    nc.vector.tensor_mul(dst[:, J], src[:], cos_sb[:, TL])
                    nc.vector.tensor_add(dst[:, J], dst[:, J], tmp[:])

                # V transpose: [64, 128] tiles -> [128, 64] into v_all
                for h in range(HPC):
                    hs = slice(h * 64, (h + 1) * 64)
                    for tt in range(4):
                        ps_t = ppool.tile([128, 64], F32R, tag="pD")
                        nc.tensor.transpose(
                            ps_t[:, :],
                            vT[hs, tt * 128:(tt + 1) * 128],
                            id_sb[hs, hs],
                        )
                        nc.scalar.copy(
                            v_all[:, b, h, (j % 4) * 4 + tt, 0:64], ps_t[:]
                        )

            # ---- Phase B + A2A, head-major for comm overlap ----
            a2a_in = [dpool.tile([N_CORES, 64, 512], BF16, tag=f"a2a_in{h}",
                                 name=f"a2a_in{h}")
                      for h in range(HPC)]
            a2a_out = [dpool.tile([N_CORES, 64, 512], BF16, tag=f"a2a_out{h}",
                                  name=f"a2a_out{h}")
                       for h in range(HPC)]

            for h in range(HPC):
                hs = slice(h * 64, (h + 1) * 64)
                for b in range(B):
                    base = b * T
                    for qc in range(T // QCHUNK):
                        q0 = qc * QCHUNK
                        n_full = q0 // 128
                        attv = ppool.tile([65, 512], F32, tag="pB")
                        for kt in range(n_full + 4):
                            k0 = kt * 128
                            ps_s = ppool.tile([128, 512], F32, tag="pA")
                            nc.tensor.matmul(
                                ps_s[:],
                                krope[hs, base + k0:base + k0 + 128],
                                qrope[hs, base + q0:base + q0 + 512],
                                start=True, stop=True,
                            )
                            ae = wpool.tile([128, 512], F32R, tag="attexp")
                            if kt < n_full:
                                nc.scalar.activation(
                                    ae[:], ps_s[:],
                                    mybir.ActivationFunctionType.Exp)
                            else:
                                v = kt - n_full
                                nc.scalar.activation(
                                    ae[:, v * 128:512], ps_s[:, v * 128:512],
                                    mybir.ActivationFunctionType.Exp)
                                nc.vector.tensor_mul(
                                    ae[:, v * 128:(v + 1) * 128],
                                    ae[:, v * 128:(v + 1) * 128],
                                    tri_sb[:],
                                )
                                if v > 0:
                                    nc.vector.memset(ae[:, 0:v * 128].bitcast(F32), 0.0)
                            nc.tensor.matmul(
                                attv[:], v_all[:, b, h, kt, :], ae[:],
                                start=(kt == 0), stop=(kt == n_full + 3),
                            )
                        # normalize: recip of row-sums, broadcast, scale
                        rcp = wpool.tile([65, 512], F32, tag="rcp")
                        nc.vector.reciprocal(rcp[64:65, :], attv[64:65, :])
                        # partition_broadcast needs its source at partition 0
                        # of the tile; DMA moves the row (engines can't).
                        rcp0 = wpool.tile([1, 512], F32, tag="rcp0")
                        nc.sync.dma_start(rcp0[:], rcp[64:65, :])
                        bcast = wpool.tile([64, 512], F32, tag="bcast")
                        nc.gpsimd.partition_broadcast(bcast[:], rcp0[:])
                        nc.vector.tensor_mul(
                            att_out[:, h, base + q0:base + q0 + 512],
                            attv[0:64, :], bcast[:],
                        )
                # A2A for this head's att columns
                nc.sync.dma_start(
                    a2a_in[h][:].transpose([1, 0, 2]),
                    att_out[:, h, :].rearrange("p (s q) -> p s q", s=N_CORES),
                )
                nc.gpsimd.collective_compute(
                    "AllToAll", mybir.AluOpType.bypass,
                    replica_groups=[list(range(N_CORES))],
                    ins=[a2a_in[h].opt()],
                    outs=[a2a_out[h].opt()],
                )
                nc.sync.dma_start(
                    att_all[hs, :, :],
                    a2a_out[h][:].transpose([1, 0, 2]),
                )

            if debug:
                nc.sync.dma_start(dbg_qrope[:], qrope[:].bitcast(F32))
                nc.sync.dma_start(dbg_krope[:], krope[:].bitcast(F32))
                nc.sync.dma_start(dbg_vall[:],
                                  v_all[:].rearrange("p a b c d -> p (a b c d)").bitcast(F32))
                nc.gpsimd.dma_start(dbg_attout[:],
                                    att_out[:].rearrange("p a b -> p (a b)"))
                nc.gpsimd.dma_start(dbg_attall[:],
                                    att_all[:].rearrange("p a b -> p (a b)"))

            # ---- Phase C: row-parallel output projection ----
            for oc in range(2):
                wo_sb = wpool.tile([128, NF, 512], BF16, tag="wo", bufs=1)
                nc.gpsimd.dma_start(
                    wo_sb[:],
                    wo_t[:, oc * 512:(oc + 1) * 512]
                    .rearrange("(c p) o -> c p o", p=128)
                    .transpose([1, 0, 2]),
                )
                for s in range(4):
                    ps_o = ppool.tile([128, 512], F32, tag="pC")
                    for c in range(N_CORES):
                        nc.tensor.matmul(
                            ps_o[:],
                            att_all[:, c, s * 128:(s + 1) * 128],
                            wo_sb[:, c, :],
                            start=(c == 0), stop=(c == N_CORES - 1),
                        )
                    o_sb = wpool.tile([128, 512], F32, tag="osb")
                    nc.scalar.copy(o_sb[:], ps_o[:])
                    nc.sync.dma_start(
                        out[s * 128:(s + 1) * 128, oc * 512:(oc + 1) * 512],
                        o_sb[:],
                    )
    nc.compile()
    return nc


def _prep_in_maps(x, wq, wk, wv, wo, cos, sin, mask):
    xt = np.ascontiguousarray(x.reshape(BT, D).T).astype(np.float32)
    wo_t = np.ascontiguousarray(wo.T).astype(np.float32)
    cos2 = np.ascontiguousarray(np.tile(cos.T, (HPC, 1))).astype(np.float32)
    sin2 = np.ascontiguousarray(np.tile(sin.T, (HPC, 1))).astype(np.float32)
    rot2t = np.ascontiguousarray(_rot_matrix().T)
    ident = np.eye(128, dtype=np.float32)
    trimask = np.ascontiguousarray(mask[0, 0, :128, :128].T).astype(np.float32)
    scale = HD ** -0.5
    in_maps = []
    for c in range(N_CORES):
        rows = slice(c * 128, (c + 1) * 128)
        in_maps.append({
            "xt": xt,
            "wq_t": np.ascontiguousarray((wq[rows, :] * scale).T).astype(np.float32),
            "wk_t": np.ascontiguousarray(wk[rows, :].T).astype(np.float32),
            "wv_t": np.ascontiguousarray(wv[rows, :].T).astype(np.float32),
            "wo_t": wo_t,
            "cos2": cos2,
            "sin2": sin2,
            "rot2t": rot2t,
            "ident": ident,
            "trimask": trimask,
        })
    return in_maps


def kernel(x, wq, wk, wv, wo, cos, sin, mask, _trace=False, _debug=False):
    key = ("nc", _debug)
    if key not in _CACHE:
        _CACHE[key] = build(debug=_debug)
    nc = _CACHE[key]
    in_maps = _prep_in_maps(x, wq, wk, wv, wo, cos, sin, mask)
    res = bass_utils.run_bass_kernel_spmd(
        nc, in_maps, core_ids=list(range(N_CORES)), trace=_trace)
    _CACHE["last_result"] = res
    full = np.concatenate([res.results[c]["out"] for c in range(N_CORES)], axis=0)
    return full.reshape(B, T, D).astype(np.float32)


# revision 10
# speedup vs baseline: 1.3916x; 1.3916x over previous
"""Multi-head causal attention with RoPE on 8 TRN2 NeuronCores.

Problem: B=2, T=2048, D=1024, H=16 heads, head_dim=64.
  out = softmax(mask(rope(x@Wq.T) @ rope(x@Wk.T).T / 8)) @ (x@Wv.T) @ Wo.T

Sharding: tensor-parallel over heads. Core c owns heads {2c, 2c+1}:
  - computes Q/K/V projections for its 2 heads over all 4096 tokens
    (x pre-transposed on host to [1024, 4096], weights pre-sliced and
    pre-transposed; the 1/sqrt(hd) scale is folded into Wq),
  - RoPE via a block-diagonal rotation matmul + cos/sin elementwise,
  - causal flash-style attention in transposed layout (scores^T [k, q]
    tiles, exp on ScalarE, lower-triangle tiles only, row-sums via an
    appended ones-column on V),
  - an AllToAll (2 x 1MB, split by local head) redistributes attention
    outputs from head-sharded to row-sharded,
  - row-parallel output projection: core c computes output rows
    [512c, 512(c+1)) of the flattened [4096, 1024] output.

All matmuls run as float32r (fp32 storage, TF32-like multiply, full PE
rate at N>=256); accumulation is fp32 in PSUM.
"""
import sys

sys.path.insert(0, "/opt/trn_rl_repo")

import numpy as np

from concourse import bacc, mybir, tile
from concourse import bass_utils

N_CORES = 8
B, T, D, H = 2, 2048, 1024, 16
HD = D // H              # 64
HPC = H // N_CORES       # 2 heads per core
BT = B * T               # 4096
NF = D // 128            # 8 feature chunks
NTC = BT // 512          # 8 t-chunks of 512
QCHUNK = 512
ROWS_PER_CORE = BT // N_CORES  # 512 output rows per core

F32 = mybir.dt.float32
F32R = mybir.dt.float32r
BF16 = mybir.dt.bfloat16

_CACHE = {}


def _rot_matrix():
    """R2 = blockdiag(R, R), R@u = rotate_half(u) per 64-dim head."""
    half = HD // 2
    R = np.zeros((HD, HD), dtype=np.float32)
    for i in range(half):
        R[i, i + half] = -1.0
        R[i + half, i] = 1.0
    R2 = np.zeros((2 * HD, 2 * HD), dtype=np.float32)
    R2[:HD, :HD] = R
    R2[HD:, HD:] = R
    return R2


def build(debug=False):
    nc = bacc.Bacc("TRN2", target_bir_lowering=False, debug=False,
                   num_devices=N_CORES)

    # ---- DRAM parameters (per-core shards, host-prepped layouts) ----
    xt = nc.declare_dram_parameter("xt", [D, BT], F32, isOutput=False)
    wq_t = nc.declare_dram_parameter("wq_t", [D, 128], F32, isOutput=False)
    wk_t = nc.declare_dram_parameter("wk_t", [D, 128], F32, isOutput=False)
    wv_t = nc.declare_dram_parameter("wv_t", [D, 128], F32, isOutput=False)
    wo_t = nc.declare_dram_parameter("wo_t", [D, D], F32, isOutput=False)
    cos2 = nc.declare_dram_parameter("cos2", [128, T], F32, isOutput=False)
    sin2 = nc.declare_dram_parameter("sin2", [128, T], F32, isOutput=False)
    rot2t = nc.declare_dram_parameter("rot2t", [128, 128], F32, isOutput=False)
    ident = nc.declare_dram_parameter("ident", [128, 128], F32, isOutput=False)
    trimask = nc.declare_dram_parameter("trimask", [128, 128], F32, isOutput=False)
    out = nc.declare_dram_parameter("out", [ROWS_PER_CORE, D], F32, isOutput=True)
    if debug:
        dbg_qrope = nc.declare_dram_parameter("dbg_qrope", [128, BT], F32, isOutput=True)
        dbg_krope = nc.declare_dram_parameter("dbg_krope", [128, BT], F32, isOutput=True)
        dbg_vall = nc.declare_dram_parameter("dbg_vall", [128, B * HPC * 16 * 65], F32, isOutput=True)
        dbg_attout = nc.declare_dram_parameter("dbg_attout", [64, HPC * BT], F32, isOutput=True)
        dbg_attall = nc.declare_dram_parameter("dbg_attall", [128, N_CORES * QCHUNK], F32, isOutput=True)

    with tile.TileContext(nc) as tc, nc.allow_low_precision(reason="f32r compute"):
        with (
            tc.tile_pool(name="consts", bufs=1) as cpool,
            tc.tile_pool(name="work", bufs=2) as wpool,
            tc.tile_pool(name="psum", bufs=1, space="PSUM") as ppool,
            tc.tile_pool(name="dram", bufs=1, space="DRAM") as dpool,
        ):
            # ---- load constants ----
            wq_sb = cpool.tile([128, NF, 128], F32R, tag="wq")
            wk_sb = cpool.tile([128, NF, 128], F32R, tag="wk")
            wv_sb = cpool.tile([128, NF, 128], F32R, tag="wv")
            for w_sb, w_dram in ((wq_sb, wq_t), (wk_sb, wk_t), (wv_sb, wv_t)):
                nc.sync.dma_start(
                    w_sb[:],
                    w_dram[:].rearrange("(c p) m -> c p m", p=128)
                    .transpose([1, 0, 2]).bitcast(F32R),
                )
            rot_sb = cpool.tile([128, 128], F32R, tag="rot")
            nc.sync.dma_start(rot_sb[:], rot2t[:].bitcast(F32R))
            id_sb = cpool.tile([128, 128], F32R, tag="ident")
            nc.sync.dma_start(id_sb[:], ident[:].bitcast(F32R))
            tri_sb = cpool.tile([128, 128], F32R, tag="tri")
            nc.sync.dma_start(tri_sb[:], trimask[:].bitcast(F32R))
            cos_sb = cpool.tile([128, T], F32R, tag="cos")
            nc.sync.dma_start(cos_sb[:], cos2[:].bitcast(F32R))
            sin_sb = cpool.tile([128, T], F32R, tag="sin")
            nc.sync.dma_start(sin_sb[:], sin2[:].bitcast(F32R))

            # persistent activations
            qrope = cpool.tile([128, BT], F32R, tag="qrope")
            krope = cpool.tile([128, BT], F32R, tag="krope")
            # V per (b, h): [128 t-part, 16 t-tiles, 65] (col 64 = ones)
            v_all = cpool.tile([128, B, HPC, T // 128, 65], F32R, tag="v_all")
            nc.vector.memset(v_all[:, :, :, :, 64].bitcast(F32), 1.0)
            # attention output, head-major [64, B*T] per local head
            att_out = cpool.tile([64, HPC, BT], BF16, tag="att_out")
            # post-A2A gathered att rows for my q-range [128, 8 src, 512]
            att_all = cpool.tile([128, N_CORES, QCHUNK], BF16, tag="att_all")

            # ---- Phase A: projections + RoPE + V transpose ----
            for j in range(NTC):
                b = j // 4
                tl = (j % 4) * 512            # t offset within batch
                J = slice(j * 512, (j + 1) * 512)
                TL = slice(tl, tl + 512)

                xt_sb = wpool.tile([128, NF, 512], F32R, tag="xt")
                nc.sync.dma_start(
                    xt_sb[:],
                    xt[:, J].rearrange("(c p) t -> c p t", p=128)
                    .transpose([1, 0, 2]).bitcast(F32R),
                )

                ps_q = ppool.tile([128, 512], F32, tag="pA", bufs=3)
                ps_k = ppool.tile([128, 512], F32, tag="pB", bufs=2)
                ps_v = ppool.tile([128, 512], F32, tag="pC", bufs=2)
                for f in range(NF):
                    st, sp = (f == 0), (f == NF - 1)
                    nc.tensor.matmul(ps_q[:], wq_sb[:, f, :], xt_sb[:, f, :],
                                     start=st, stop=sp)
                    nc.tensor.matmul(ps_k[:], wk_sb[:, f, :], xt_sb[:, f, :],
                                     start=st, stop=sp)
                    nc.tensor.matmul(ps_v[:], wv_sb[:, f, :], xt_sb[:, f, :],
                                     start=st, stop=sp)

                # copy projections to SBUF
                qT = wpool.tile([128, 512], F32R, tag="qT")
                kT = wpool.tile([128, 512], F32R, tag="kT")
                vT = wpool.tile([128, 512], F32R, tag="vT")
                nc.vector.tensor_copy(qT[:], ps_q[:])
                nc.vector.tensor_copy(kT[:], ps_k[:])
                nc.vector.tensor_copy(vT[:], ps_v[:])

                # RoPE: rope(u) = u*cos + (R2@u)*sin
                for src, dst in ((qT, qrope), (kT, krope)):
                    ps_r = ppool.tile([128, 512], F32, tag="pD", bufs=1)
                    nc.tensor.matmul(ps_r[:], rot_sb[:], src[:],
                                     start=True, stop=True)
                    tmp = wpool.tile([128, 512], F32R, tag="ropetmp")
                    nc.vector.tensor_mul(tmp[:], ps_r[:], sin_sb[:, TL])
                    nc.vector.tensor_mul(dst[:, J], src[:], cos_sb[:, TL])
                    nc.vector.tensor_add(dst[:, J], dst[:, J], tmp[:])

                # V transpose: [64, 128] tiles -> [128, 64] into v_all
                for h in range(HPC):
                    hs = slice(h * 64, (h + 1) * 64)
                    for tt in range(4):
                        ps_t = ppool.tile([128, 64], F32R, tag="pD", bufs=1)
                        nc.tensor.transpose(
                            ps_t[:, :],
                            vT[hs, tt * 128:(tt + 1) * 128],
                            id_sb[hs, hs],
                        )
                        nc.scalar.copy(
                            v_all[:, b, h, (j % 4) * 4 + tt, 0:64], ps_t[:]
                        )

            # ---- Phase B + A2A, head-major for comm overlap ----
            a2a_in = [dpool.tile([N_CORES, 64, 512], BF16, tag=f"a2a_in{h}",
                                 name=f"a2a_in{h}")
                      for h in range(HPC)]
            a2a_out = [dpool.tile([N_CORES, 64, 512], BF16, tag=f"a2a_out{h}",
                                  name=f"a2a_out{h}")
                       for h in range(HPC)]

            def scores_mm(hs, base, q0, kt):
                k0 = kt * 128
                ps_s = ppool.tile([128, 512], F32, tag="pA", bufs=3, name="ps_s")
                nc.tensor.matmul(
                    ps_s[:],
                    krope[hs, base + k0:base + k0 + 128],
                    qrope[hs, base + q0:base + q0 + 512],
                    start=True, stop=True,
                )
                return ps_s

            def exp_mask(ps_s, n_full, kt):
                ae = wpool.tile([128, 512], F32R, tag="attexp", bufs=4,
                                name="ae")
                if kt < n_full:
                    nc.scalar.activation(
                        ae[:], ps_s[:], mybir.ActivationFunctionType.Exp)
                else:
                    v = kt - n_full
                    nc.scalar.activation(
                        ae[:, v * 128:512], ps_s[:, v * 128:512],
                        mybir.ActivationFunctionType.Exp)
                    nc.vector.tensor_mul(
                        ae[:, v * 128:(v + 1) * 128],
                        ae[:, v * 128:(v + 1) * 128],
                        tri_sb[:],
                    )
                    if v > 0:
                        nc.vector.memset(ae[:, 0:v * 128].bitcast(F32), 0.0)
                return ae

            for h in range(HPC):
                hs = slice(h * 64, (h + 1) * 64)
                for b in range(B):
                    base = b * T
                    for qc in range(T // QCHUNK):
                        q0 = qc * QCHUNK
                        n_full = q0 // 128
                        n_kt = n_full + 4
                        attv = ppool.tile([65, 512], F32, tag="pB", bufs=2)
                        # software-pipelined emission: scores one tile ahead
                        ps_s = scores_mm(hs, base, q0, 0)
                        for kt in range(n_kt):
                            ae = exp_mask(ps_s, n_full, kt)
                            if kt + 1 < n_kt:
                                ps_s = scores_mm(hs, base, q0, kt + 1)
                            nc.tensor.matmul(
                                attv[:], v_all[:, b, h, kt, :], ae[:],
                                start=(kt == 0), stop=(kt == n_kt - 1),
                            )
                        # normalize: broadcast row-sums, recip on 64 lanes
                        sumrow = wpool.tile([65, 512], F32, tag="sumrow")
                        nc.vector.tensor_copy(sumrow[64:65, :], attv[64:65, :])
                        # partition_broadcast needs its source at partition 0
                        # of the tile; DMA moves the row (engines can't).
                        sum0 = wpool.tile([1, 512], F32, tag="sum0")
                        nc.sync.dma_start(sum0[:], sumrow[64:65, :])
                        bsum = wpool.tile([64, 512], F32, tag="bsum")
                        nc.gpsimd.partition_broadcast(bsum[:], sum0[:])
                        brcp = wpool.tile([64, 512], F32, tag="brcp")
                        nc.vector.reciprocal(brcp[:], bsum[:])
                        nc.vector.tensor_mul(
                            att_out[:, h, base + q0:base + q0 + 512],
                            attv[0:64, :], brcp[:],
                        )
                # A2A for this head's att columns
                nc.sync.dma_start(
                    a2a_in[h][:].transpose([1, 0, 2]),
                    att_out[:, h, :].rearrange("p (s q) -> p s q", s=N_CORES),
                )
                nc.gpsimd.collective_compute(
                    "AllToAll", mybir.AluOpType.bypass,
                    replica_groups=[list(range(N_CORES))],
                    ins=[a2a_in[h].opt()],
                    outs=[a2a_out[h].opt()],
                )
                nc.sync.dma_start(
                    att_all[hs, :, :],
                    a2a_out[h][:].transpose([1, 0, 2]),
                )

            if debug:
                nc.sync.dma_start(dbg_qrope[:], qrope[:].bitcast(F32))
                nc.sync.dma_start(dbg_krope[:], krope[:].bitcast(F32))
                nc.sync.dma_start(dbg_vall[:],
                                  v_all[:].rearrange("p a b c d -> p (a b c d)").bitcast(F32))
                nc.gpsimd.dma_start(dbg_attout[:],
                                    att_out[:].rearrange("p a b -> p (a b)"))
                nc.gpsimd.dma_start(dbg_attall[:],
                                    att_all[:].rearrange("p a b -> p (a b)"))

            # ---- Phase C: row-parallel output projection ----
            for oc in range(2):
                wo_sb = wpool.tile([128, NF, 512], BF16, tag="wo", bufs=1)
                nc.gpsimd.dma_start(
                    wo_sb[:],
                    wo_t[:, oc * 512:(oc + 1) * 512]
                    .rearrange("(c p) o -> c p o", p=128)
                    .transpose([1, 0, 2]),
                )
                for s in range(4):
                    ps_o = ppool.tile([128, 512], F32, tag="pC", bufs=2)
                    for c in range(N_CORES):
                        nc.tensor.matmul(
                            ps_o[:],
                            att_all[:, c, s * 128:(s + 1) * 128],
                            wo_sb[:, c, :],
                            start=(c == 0), stop=(c == N_CORES - 1),
                        )
                    o_sb = wpool.tile([128, 512], F32, tag="osb")
                    nc.scalar.copy(o_sb[:], ps_o[:])
                    nc.sync.dma_start(
                        out[s * 128:(s + 1) * 128, oc * 512:(oc + 1) * 512],
                        o_sb[:],
                    )
    nc.compile()
    return nc


def _prep_in_maps(x, wq, wk, wv, wo, cos, sin, mask):
    xt = np.ascontiguousarray(x.reshape(BT, D).T).astype(np.float32)
    wo_t = np.ascontiguousarray(wo.T).astype(np.float32)
    cos2 = np.ascontiguousarray(np.tile(cos.T, (HPC, 1))).astype(np.float32)
    sin2 = np.ascontiguousarray(np.tile(sin.T, (HPC, 1))).astype(np.float32)
    rot2t = np.ascontiguousarray(_rot_matrix().T)
    ident = np.eye(128, dtype=np.float32)
    trimask = np.ascontiguousarray(mask[0, 0, :128, :128].T).astype(np.float32)
    scale = HD ** -0.5
    in_maps = []
    for c in range(N_CORES):
        rows = slice(c * 128, (c + 1) * 128)
        in_maps.append({
            "xt": xt,
            "wq_t": np.ascontiguousarray((wq[rows, :] * scale).T).astype(np.float32),
            "wk_t": np.ascontiguousarray(wk[rows, :].T).astype(np.float32),
            "wv_t": np.ascontiguousarray(wv[rows, :].T).astype(np.float32),
            "wo_t": wo_t,
            "cos2": cos2,
            "sin2": sin2,
            "rot2t": rot2t,
            "ident": ident,
            "trimask": trimask,
        })
    return in_maps


def kernel(x, wq, wk, wv, wo, cos, sin, mask, _trace=False, _debug=False):
    key = ("nc", _debug)
    if key not in _CACHE:
        _CACHE[key] = build(debug=_debug)
    nc = _CACHE[key]
    in_maps = _prep_in_maps(x, wq, wk, wv, wo, cos, sin, mask)
    res = bass_utils.run_bass_kernel_spmd(
        nc, in_maps, core_ids=list(range(N_CORES)), trace=_trace)
    _CACHE["last_result"] = res
    full = np.concatenate([res.results[c]["out"] for c in range(N_CORES)], axis=0)
    return full.reshape(B, T, D).astype(np.float32)
